# revision 31
# baseline (speedup 1.0000x reference)
"""3-layer GCN (GCNConv+BN+ReLU x2, GCNConv) on 8 Trainium2 NeuronCores.

Strategy: shard nodes across the 8 cores (graph parallel). Per layer:
  1. dense matmul H = Z @ W on the core's node shard (weights replicated,
     BN scale folded into W on the host),
  2. AllGather the node-feature table (bf16) so every core can gather
     messages from any source node,
  3. sparse propagation A_norm @ H via bulk row-gather (dma_gather) +
     per-128-edge-chunk selection-matrix matmuls accumulated in PSUM,
  4. epilogue: +bias, ReLU (layers 1/2).

Edges are grouped on the host by (core of dst, dst block of 128, src table
quarter) — the quarter matches the AllGather sub-collective layout, so each
gather depends only on its own sub-AllGather slice (AG overlaps the sparse
phase), and quarter-relative indices stay inside int16 range. Every
(block, quarter) group is padded to a uniform chunk count so a single SPMD
program serves all 8 cores. Self-loops are appended exactly as in the
reference. norm = dinv[src]*dinv[dst] is folded away: dinv[src] scales the
table rows (dense epilogues), dinv[dst] scales the PSUM in the sparse
epilogues, so the selection matrices are pure one-hots (pad slots use
dstlocal=-1 so they match no column).
"""

from dataclasses import dataclass

import ml_dtypes
import numpy as np

import concourse.bass as bass
import concourse.tile as tile
from concourse import bacc, mybir
from concourse.bass_utils import run_bass_kernel_spmd

BF16 = mybir.dt.bfloat16
F32 = mybir.dt.float32
I16 = mybir.dt.int16
P = 128
NPBF = ml_dtypes.bfloat16


@dataclass(frozen=True)
class GCNConfig:
    ncores: int = 8
    n: int = 50000       # real nodes
    npad: int = 50176    # ncores * shard
    shard: int = 6272    # nodes per core = nb * 128
    nb: int = 49         # node blocks per core
    kin: int = 10        # input-feature k-tiles (IN = kin * 128)
    hid: int = 256       # hidden dim (2 feature blocks of 128)
    out: int = 500       # output dim
    split: int = 32768   # src index lo/hi split (int16 gather indices)
    eps: float = 1e-5
    max_gather_chunks: int = 8   # chunks (of 128 idxs) per dma_gather
    dma_scratch: int = 16384     # SWDGE descriptor-ring carveout bytes
    single_packet: bool = True   # dma_gather packetization
    shared_tab: bool = False     # AllGather outputs in Shared addr space
    host_sel: bool = False       # DMA host-built selection matrices
                                 # instead of building them on DVE
    ag_splits: int = 4           # split each AllGather into S sub-collectives
    swdge_queues: int = 4        # SWDGE queues to round-robin gathers over
    # ablation switches (timing experiments only — results become wrong)
    abl_no_ag: bool = False      # replace AllGather with a local DMA copy
    abl_no_gather: bool = False  # replace dma_gather with DVE memset
    abl_no_mm: bool = False      # skip sel build + SpMM matmuls


FULL_CFG = GCNConfig()


# ---------------------------------------------------------------------------
# host-side preprocessing
# ---------------------------------------------------------------------------


def _wrap_idx(tok: np.ndarray) -> np.ndarray:
    """[..., n] int16 token list -> [..., 16, n//16] wrapped layout.

    dma_gather expects token i at [i % 16, i // 16] in 16 partitions.
    """
    lead = tok.shape[:-1]
    n = tok.shape[-1]
    w = tok.reshape(*lead, n // 16, 16)
    return np.swapaxes(w, -1, -2)


def preprocess(cfg: GCNConfig, x, edge_index, W1, b1, g1, be1, m1, v1,
               W2, b2, g2, be2, m2, v2, W3, b3):
    n = cfg.n
    C, NB = cfg.ncores, cfg.nb
    hid, kin = cfg.hid, cfg.kin
    k2 = hid // P

    x = np.asarray(x, dtype=np.float32)
    edge_index = np.asarray(edge_index)
    loops = np.arange(n, dtype=np.int64)
    src = np.concatenate([edge_index[0].astype(np.int64), loops])
    dst = np.concatenate([edge_index[1].astype(np.int64), loops])

    deg = np.bincount(dst, minlength=n).astype(np.float32)
    dinv = np.where(deg > 0, 1.0 / np.sqrt(deg), 0.0).astype(np.float32)
    norm = (dinv[src] * dinv[dst]).astype(np.float32)

    core = dst // cfg.shard
    blk = (dst % cfg.shard) // P
    dl = (dst % P).astype(np.float32)
    # gather-table row of a source node: with split AllGathers the table is
    # laid out (subrange, core, row) so each sub-collective output is a
    # contiguous row block. Edges are grouped by subrange so each gather
    # depends only on its own sub-collective slice.
    S = cfg.ag_splits
    sub = cfg.shard // S
    qrows = C * sub
    s_core = src // cfg.shard
    s_loc = src % cfg.shard
    q = s_loc // sub
    gval = (s_core * sub + (s_loc % sub)).astype(np.int16)  # quarter-relative

    gid = (core * NB + blk) * S + q
    ngroups = C * NB * S
    counts = np.bincount(gid, minlength=ngroups)
    cS = counts.reshape(C, NB, S)
    m_q = [max(1, int(np.ceil(cS[:, :, j].max() / P))) for j in range(S)]
    M = sum(m_q)
    moff = np.concatenate([[0], np.cumsum(m_q)])  # chunk offsets per quarter

    cap = np.empty(ngroups, dtype=np.int64)
    for j in range(S):
        cap[j::S] = m_q[j] * P
    base = np.concatenate([[0], np.cumsum(cap)])[:-1]

    order = np.argsort(gid, kind="stable")
    gs = gid[order]
    start_sorted = np.concatenate([[0], np.cumsum(counts)])[:-1]
    rank = np.arange(len(gs)) - start_sorted[gs]
    slot = base[gs] + rank

    tot = int(cap.sum())
    s_idx = np.zeros(tot, dtype=np.int16)
    s_dl = np.full(tot, -1.0, dtype=np.float32)   # pad: match no dst column
    s_idx[slot] = gval[order]
    s_dl[slot] = dl[order]

    s_idx3 = s_idx.reshape(C, NB, M * P)
    w_all = np.concatenate(
        [_wrap_idx(s_idx3[..., moff[j] * P: moff[j + 1] * P])
         for j in range(S)], axis=-1)               # [C, NB, 16, M*8]
    w_all = np.tile(w_all, (1, 1, 8, 1))            # [C, NB, 128, M*8]
    gidx_dram = np.ascontiguousarray(
        np.moveaxis(w_all, 2, 1)).reshape(C, P, NB * M * 8)

    # per-edge dst-local index, [C, P(edge slot), NB*M] layout
    dl4 = s_dl.reshape(C, NB, M, P)
    meta_dram = np.ascontiguousarray(
        np.moveaxis(dl4, 3, 1)).reshape(C, P, NB * M)

    # host-built selection matrices: sel[e, d] = (dstlocal_e == d)
    sel5 = (dl4[..., None] == np.arange(P, dtype=np.float32)).astype(NPBF)
    selv_dram = np.ascontiguousarray(
        np.moveaxis(sel5, 3, 1)).reshape(C, P, NB * M * P)

    # per-node dinv, laid out [P, NB] per core (column nb = block's nodes)
    dinv_pad = np.zeros(cfg.npad, dtype=np.float32)
    dinv_pad[:n] = dinv
    dinvd_dram = np.ascontiguousarray(
        dinv_pad.reshape(C, NB, P).transpose(0, 2, 1))  # [C, P, NB]

    # dense inputs: x^T tiles [c, nb, ci, k*nn]
    x_pad = np.zeros((cfg.npad, kin * P), dtype=np.float32)
    x_pad[:n] = x
    xt = x_pad.reshape(C, NB, P, kin, P).transpose(0, 1, 4, 3, 2)
    xT_dram = np.ascontiguousarray(xt).astype(NPBF).reshape(C, NB, P, kin * P)

    # weights with BN scale folded in; [ci, k*co] layout
    s1 = (np.asarray(g1) / np.sqrt(np.asarray(v1) + cfg.eps)).astype(np.float32)
    s2 = (np.asarray(g2) / np.sqrt(np.asarray(v2) + cfg.eps)).astype(np.float32)
    W1p = np.asarray(W1, np.float32) * s1[None, :]
    W2p = np.asarray(W2, np.float32) * s2[None, :]

    def wtiles(w, ktiles, cout):
        t = w.reshape(ktiles, P, cout).transpose(1, 0, 2)
        return np.ascontiguousarray(t).astype(NPBF).reshape(P, ktiles * cout)

    w1_dram = wtiles(W1p, kin, hid)
    w2_dram = wtiles(W2p, k2, hid)
    w3_dram = wtiles(np.asarray(W3, np.float32), k2, cfg.out)

    bias1 = ((np.asarray(b1) - np.asarray(m1)) * s1 + np.asarray(be1)).astype(np.float32)
    bias2 = ((np.asarray(b2) - np.asarray(m2)) * s2 + np.asarray(be2)).astype(np.float32)
    bias1_bc = np.ascontiguousarray(np.broadcast_to(bias1, (P, hid)))
    bias2_bc = np.ascontiguousarray(np.broadcast_to(bias2, (P, hid)))
    b3_bc = np.ascontiguousarray(
        np.broadcast_to(np.asarray(b3, np.float32), (P, cfg.out)))

    iota = np.ascontiguousarray(
        np.broadcast_to(np.arange(P, dtype=np.float32), (P, P)))
    ident = np.eye(P, dtype=np.float32).astype(NPBF)

    in_maps = []
    for c in range(C):
        extra = ({"selv": selv_dram[c]} if cfg.host_sel
                 else {"meta": meta_dram[c]})
        in_maps.append({
            **extra,
            "xT": xT_dram[c],
            "gidx": gidx_dram[c],
            "dinvd": dinvd_dram[c],
            "w1": w1_dram,
            "w2": w2_dram,
            "w3": w3_dram,
            "bias1": bias1_bc,
            "bias2": bias2_bc,
            "b3bc": b3_bc,
            "iota": iota,
            "ident": ident,
        })
    return in_maps, tuple(m_q)


# ---------------------------------------------------------------------------
# device kernel
# ---------------------------------------------------------------------------


def build_nc(cfg: GCNConfig, m_q: tuple):
    C, NB = cfg.ncores, cfg.nb
    hid, kin, outd = cfg.hid, cfg.kin, cfg.out
    k2 = hid // P
    S = cfg.ag_splits
    sub = cfg.shard // S
    qrows = C * sub
    M = sum(m_q)
    moff = [0]
    for m in m_q:
        moff.append(moff[-1] + m)

    nc = bacc.Bacc("TRN2", target_bir_lowering=False, debug=False,
                   num_devices=C, dynamic_dma_scratch_size=cfg.dma_scratch,
                   num_swdge_queues=cfg.swdge_queues)

    xT = nc.dram_tensor("xT", [NB, P, kin * P], BF16, kind="ExternalInput")
    gidx = nc.dram_tensor("gidx", [P, NB * M * 8], I16, kind="ExternalInput")
    if cfg.host_sel:
        selv = nc.dram_tensor("selv", [P, NB * M * P], BF16,
                              kind="ExternalInput")
    else:
        meta = nc.dram_tensor("meta", [P, NB * M], F32,
                              kind="ExternalInput")
    w1 = nc.dram_tensor("w1", [P, kin * hid], BF16, kind="ExternalInput")
    w2 = nc.dram_tensor("w2", [P, k2 * hid], BF16, kind="ExternalInput")
    w3 = nc.dram_tensor("w3", [P, k2 * outd], BF16, kind="ExternalInput")
    dinvd = nc.dram_tensor("dinvd", [P, NB], F32, kind="ExternalInput")
    bias1 = nc.dram_tensor("bias1", [P, hid], F32, kind="ExternalInput")
    bias2 = nc.dram_tensor("bias2", [P, hid], F32, kind="ExternalInput")
    b3bc = nc.dram_tensor("b3bc", [P, outd], F32, kind="ExternalInput")
    iota = nc.dram_tensor("iota", [P, P], F32, kind="ExternalInput")
    ident = nc.dram_tensor("ident", [P, P], BF16, kind="ExternalInput")
    out = nc.dram_tensor("out", [cfg.shard, outd], F32, kind="ExternalOutput")

    groups = [list(range(C))]

    with tile.TileContext(nc) as tc:
        with (
            tc.tile_pool(name="const", bufs=1) as constp,
            tc.tile_pool(name="zfeat", bufs=1) as zfeatp,
            tc.tile_pool(name="dram", bufs=2, space="DRAM") as dramp,
            tc.tile_pool(name="work", bufs=3) as workp,
            tc.tile_pool(name="msg", bufs=2) as msgp,
            tc.tile_pool(name="sel", bufs=2) as selp,
            tc.tile_pool(name="psum", bufs=3, space="PSUM") as psump,
            tc.tile_pool(name="psumt", bufs=2, space="PSUM") as psumtp,
        ):
            # ---- constants ----
            w1_t = constp.tile([P, kin * hid], BF16, tag="w1")
            nc.sync.dma_start(out=w1_t[:], in_=w1[:, :])
            w2_t = constp.tile([P, k2 * hid], BF16, tag="w2")
            nc.sync.dma_start(out=w2_t[:], in_=w2[:, :])
            w3_t = constp.tile([P, k2 * outd], BF16, tag="w3")
            nc.sync.dma_start(out=w3_t[:], in_=w3[:, :])
            bias1_t = constp.tile([P, hid], F32, tag="bias1")
            nc.sync.dma_start(out=bias1_t[:], in_=bias1[:, :])
            bias2_t = constp.tile([P, hid], F32, tag="bias2")
            nc.sync.dma_start(out=bias2_t[:], in_=bias2[:, :])
            b3_t = constp.tile([P, outd], F32, tag="b3")
            nc.sync.dma_start(out=b3_t[:], in_=b3bc[:, :])
            iota_t = constp.tile([P, P], F32, tag="iota")
            nc.sync.dma_start(out=iota_t[:], in_=iota[:, :])
            ident_t = constp.tile([P, P], BF16, tag="ident")
            nc.sync.dma_start(out=ident_t[:], in_=ident[:, :])
            gidx_t = constp.tile([P, NB * M * 8], I16, tag="gidx")
            nc.sync.dma_start(out=gidx_t[:], in_=gidx[:, :])
            dinvd_t = constp.tile([P, NB], F32, tag="dinvd")
            nc.sync.dma_start(out=dinvd_t[:], in_=dinvd[:, :])
            if not cfg.host_sel:
                meta_t = constp.tile([P, NB * M], F32, tag="meta")
                nc.sync.dma_start(out=meta_t[:], in_=meta[:, :])

            # persistent feature-major activations ([feat, k * shard] layout)
            def new_zfeat(name):
                return zfeatp.tile([P, k2 * cfg.shard], BF16, tag="zf",
                                   name=name)

            def allgather(src_tile, cols, name):
                tab = dramp.tile([cfg.npad, cols], BF16, tag="tab", name=name,
                                 addr_space="Shared" if cfg.shared_tab
                                 else "Local")
                if cfg.abl_no_ag:
                    nc.sync.dma_start(out=tab[:cfg.shard, :],
                                      in_=src_tile[:, :])
                elif cfg.ag_splits > 1:
                    S = cfg.ag_splits
                    sub = cfg.shard // S
                    rows = C * sub
                    for q in range(S):
                        nc.gpsimd.collective_compute(
                            "AllGather",
                            mybir.AluOpType.bypass,
                            replica_groups=groups,
                            ins=[src_tile[q * sub:(q + 1) * sub, :].opt()],
                            outs=[tab[q * rows:(q + 1) * rows, :].opt()],
                        )
                else:
                    nc.gpsimd.collective_compute(
                        "AllGather",
                        mybir.AluOpType.bypass,
                        replica_groups=groups,
                        ins=[src_tile[:, :].opt()],
                        outs=[tab[:, :].opt()],
                    )
                return tab

            # ---- dense matmul phase: h_sh[nb*P:+P] = lhs(nb) @ W ----
            def dense(lhs_of_nb, w_tile, ktiles, cols, epilogue):
                for nb in range(NB):
                    ps = psump.tile([P, cols], F32, tag="mm", name="dps")
                    for k in range(ktiles):
                        nc.tensor.matmul(
                            out=ps[:],
                            lhsT=lhs_of_nb(nb, k),
                            rhs=w_tile[:, k * cols:(k + 1) * cols],
                            start=(k == 0),
                            stop=(k == ktiles - 1),
                        )
                    epilogue(nb, ps)

            # ---- sparse propagation: psum[dst 128, hid] = A_nb @ table ----
            gq_counter = [0]

            def next_gq():
                q = gq_counter[0] % cfg.swdge_queues
                gq_counter[0] += 1
                return q

            def spmm(tab, epilogue, lname):
                G = cfg.max_gather_chunks
                for nb in range(NB):
                    gbase = nb * M * 8
                    mbase = nb * M
                    g_all = msgp.tile([P, M, hid], BF16, tag="gall",
                                      name=f"gall_{lname}")
                    if cfg.abl_no_gather:
                        nc.vector.memset(g_all[:], 0.0)
                    else:
                        for j in range(S):
                            for g0 in range(moff[j], moff[j + 1], G):
                                g1 = min(g0 + G, moff[j + 1])
                                nc.gpsimd.dma_gather(
                                    out_ap=g_all[:, g0:g1, :],
                                    in_ap=tab[j * qrows:(j + 1) * qrows, :],
                                    idxs_ap=gidx_t[:, gbase + g0 * 8:
                                                   gbase + g1 * 8],
                                    num_idxs=(g1 - g0) * P,
                                    num_idxs_reg=(g1 - g0) * P,
                                    elem_size=hid,
                                    single_packet=cfg.single_packet,
                                    queue_num=next_gq(),
                                )
                    ps = psump.tile([P, hid], F32, tag="mm", name="sps")
                    if cfg.abl_no_mm:
                        nc.vector.memset(ps[:], 0.0)
                        # keep a data dep on the gathers so they stay live
                        nc.vector.tensor_tensor(
                            out=ps[:, :1], in0=g_all[:, 0, :1],
                            in1=g_all[:, M - 1, :1], op=mybir.AluOpType.add)
                        epilogue(nb, ps)
                        continue
                    strip = selp.tile([P, M * P], BF16, tag="selb",
                                      name="selstrip")
                    if cfg.host_sel:
                        nc.sync.dma_start(
                            out=strip[:],
                            in_=selv[:, nb * M * P:(nb + 1) * M * P])
                    else:
                        # one DVE op builds all M one-hot sel matrices:
                        # strip[e, m*128+d] = (dstlocal[e, m] == d)
                        dl_b = meta_t[:, mbase:mbase + M] \
                            .unsqueeze(-1).broadcast_to([P, M, P])
                        io_b = iota_t[:, :].unsqueeze(1) \
                            .broadcast_to([P, M, P])
                        nc.vector.tensor_tensor(
                            out=strip[:].rearrange("p (m d) -> p m d", m=M),
                            in0=dl_b, in1=io_b,
                            op=mybir.AluOpType.is_equal)
                    for m in range(M):
                        nc.tensor.matmul(
                            out=ps[:],
                            lhsT=strip[:, m * P:(m + 1) * P],
                            rhs=g_all[:, m, :],
                            start=(m == 0),
                            stop=(m == M - 1),
                        )
                    epilogue(nb, ps)

            # node-major [128 nodes, hid] sbuf tile -> feature-major zT slices
            def to_featmajor(zT_t, nb, zn):
                for f in range(k2):
                    pt = psumtp.tile([P, P], BF16, tag="pt", name="pt")
                    nc.tensor.transpose(
                        out=pt[:], in_=zn[:, f * P:(f + 1) * P],
                        identity=ident_t[:])
                    nc.scalar.copy(
                        out=zT_t[:, f * cfg.shard + nb * P:
                                 f * cfg.shard + (nb + 1) * P],
                        in_=pt[:])

            def bn_relu(nb, ps, bias_t, name):
                # z = relu(dinv[dst] * psum + bias)
                sc = workp.tile([P, hid], F32, tag="epsc", name="epsc")
                nc.vector.tensor_scalar_mul(
                    out=sc[:], in0=ps[:], scalar1=dinvd_t[:, nb:nb + 1])
                tmp = workp.tile([P, hid], F32, tag="eptmp", name="eptmp")
                nc.vector.tensor_tensor(out=tmp[:], in0=sc[:], in1=bias_t[:],
                                        op=mybir.AluOpType.add)
                zn = workp.tile([P, hid], BF16, tag="zn", name=name)
                nc.vector.tensor_scalar_max(out=zn[:], in0=tmp[:], scalar1=0.0)
                return zn

            # ================= layer 1 =================
            h_sh1 = dramp.tile([cfg.shard, hid], BF16, tag="hsh", name="h_sh1")

            def ep_dense1(nb, ps):
                hs = workp.tile([P, hid], BF16, tag="hs", name="hs1")
                nc.vector.tensor_scalar_mul(
                    out=hs[:], in0=ps[:], scalar1=dinvd_t[:, nb:nb + 1])
                nc.sync.dma_start(out=h_sh1[nb * P:(nb + 1) * P, :], in_=hs[:])

            # L1 dense streams xT tiles from DRAM
            for nb in range(NB):
                xt_t = workp.tile([P, kin * P], BF16, tag="xt", name="xt")
                nc.sync.dma_start(out=xt_t[:], in_=xT[nb, :, :])
                ps = psump.tile([P, hid], F32, tag="mm", name="dps1")
                for k in range(kin):
                    nc.tensor.matmul(
                        out=ps[:],
                        lhsT=xt_t[:, k * P:(k + 1) * P],
                        rhs=w1_t[:, k * hid:(k + 1) * hid],
                        start=(k == 0),
                        stop=(k == kin - 1),
                    )
                ep_dense1(nb, ps)

            h_tab1 = allgather(h_sh1, hid, "h_tab1")

            z1T = new_zfeat("z1T")

            def ep_spmm1(nb, ps):
                zn = bn_relu(nb, ps, bias1_t, "zn1")
                to_featmajor(z1T, nb, zn)

            spmm(h_tab1, ep_spmm1, "l1")

            # ================= layer 2 =================
            h_sh2 = dramp.tile([cfg.shard, hid], BF16, tag="hsh", name="h_sh2")

            def ep_dense2(nb, ps):
                hs = workp.tile([P, hid], BF16, tag="hs", name="hs2")
                nc.vector.tensor_scalar_mul(
                    out=hs[:], in0=ps[:], scalar1=dinvd_t[:, nb:nb + 1])
                nc.sync.dma_start(out=h_sh2[nb * P:(nb + 1) * P, :], in_=hs[:])

            dense(lambda nb, k: z1T[:, k * cfg.shard + nb * P:
                                    k * cfg.shard + (nb + 1) * P],
                  w2_t, k2, hid, ep_dense2)

            h_tab2 = allgather(h_sh2, hid, "h_tab2")

            z_sh2 = dramp.tile([cfg.shard, hid], BF16, tag="hsh", name="z_sh2")

            def ep_spmm2(nb, ps):
                zn = bn_relu(nb, ps, bias2_t, "zn2")
                zs = workp.tile([P, hid], BF16, tag="zs", name="zs2")
                nc.vector.tensor_scalar_mul(
                    out=zs[:], in0=zn[:], scalar1=dinvd_t[:, nb:nb + 1])
                nc.sync.dma_start(out=z_sh2[nb * P:(nb + 1) * P, :], in_=zs[:])

            spmm(h_tab2, ep_spmm2, "l2")

            z_tab2 = allgather(z_sh2, hid, "z_tab2")

            # ================= layer 3 =================
            p3T = new_zfeat("p3T")

            def ep_spmm3(nb, ps):
                pn = workp.tile([P, hid], BF16, tag="zn", name="pn3")
                nc.vector.tensor_scalar_mul(
                    out=pn[:], in0=ps[:], scalar1=dinvd_t[:, nb:nb + 1])
                to_featmajor(p3T, nb, pn)

            spmm(z_tab2, ep_spmm3, "l3")

            def ep_dense3(nb, ps):
                ot = workp.tile([P, outd], F32, tag="ot", name="ot")
                nc.vector.tensor_tensor(out=ot[:], in0=ps[:], in1=b3_t[:],
                                        op=mybir.AluOpType.add)
                nc.sync.dma_start(out=out[nb * P:(nb + 1) * P, :], in_=ot[:])

            dense(lambda nb, k: p3T[:, k * cfg.shard + nb * P:
                                    k * cfg.shard + (nb + 1) * P],
                  w3_t, k2, outd, ep_dense3)

    nc.compile()
    return nc


# ---------------------------------------------------------------------------
# entry point
# ---------------------------------------------------------------------------

_NC_CACHE: dict = {}


def prepare(cfg: GCNConfig, inputs: dict):
    in_maps, m_q = preprocess(cfg, **inputs)
    key = (cfg, m_q)
    if key not in _NC_CACHE:
        _NC_CACHE[key] = build_nc(cfg, m_q)
    return _NC_CACHE[key], in_maps


def run(cfg: GCNConfig, inputs: dict, **spmd_kwargs):
    nc, in_maps = prepare(cfg, inputs)
    res = run_bass_kernel_spmd(nc, in_maps, core_ids=list(range(cfg.ncores)),
                               **spmd_kwargs)
    full = np.concatenate(
        [res.results[c]["out"] for c in range(cfg.ncores)], axis=0)
    return np.ascontiguousarray(full[:cfg.n]).astype(np.float32), res


def kernel(**inputs) -> np.ndarray:
    out, _ = run(FULL_CFG, inputs)
    return out



# revision 36
# speedup vs baseline: 1.3751x; 1.3751x over previous
"""3-layer GCN (GCNConv+BN+ReLU x2, GCNConv) on 8 Trainium2 NeuronCores.

Strategy: shard nodes across the 8 cores (graph parallel). Per layer:
  1. dense matmul H = Z @ W on the core's node shard (weights replicated,
     BN scale folded into W on the host),
  2. AllGather the node-feature table (bf16) so every core can gather
     messages from any source node,
  3. sparse propagation A_norm @ H via bulk row-gather (dma_gather) +
     per-128-edge-chunk selection-matrix matmuls accumulated in PSUM,
  4. epilogue: +bias, ReLU (layers 1/2).

Edges are grouped on the host by (core of dst, dst block of 128, src table
quarter) — the quarter matches the AllGather sub-collective layout, so each
gather depends only on its own sub-AllGather slice (AG overlaps the sparse
phase), and quarter-relative indices stay inside int16 range. Every
(block, quarter) group is padded to a uniform chunk count so a single SPMD
program serves all 8 cores. Self-loops are appended exactly as in the
reference. norm = dinv[src]*dinv[dst] is folded away: dinv[src] scales the
table rows (dense epilogues), dinv[dst] scales the PSUM in the sparse
epilogues, so the selection matrices are pure one-hots (pad slots use
dstlocal=-1 so they match no column).
"""

from dataclasses import dataclass

import ml_dtypes
import numpy as np

import concourse.bass as bass
import concourse.tile as tile
from concourse import bacc, mybir
from concourse.bass_utils import run_bass_kernel_spmd

BF16 = mybir.dt.bfloat16
F32 = mybir.dt.float32
I16 = mybir.dt.int16
P = 128
NPBF = ml_dtypes.bfloat16


@dataclass(frozen=True)
class GCNConfig:
    ncores: int = 8
    n: int = 50000       # real nodes
    npad: int = 50176    # ncores * shard
    shard: int = 6272    # nodes per core = nb * 128
    nb: int = 49         # node blocks per core
    kin: int = 10        # input-feature k-tiles (IN = kin * 128)
    hid: int = 256       # hidden dim (2 feature blocks of 128)
    out: int = 500       # output dim
    split: int = 32768   # src index lo/hi split (int16 gather indices)
    eps: float = 1e-5
    max_gather_chunks: int = 8   # chunks (of 128 idxs) per dma_gather
    dma_scratch: int = 16384     # SWDGE descriptor-ring carveout bytes
    single_packet: bool = True   # dma_gather packetization
    shared_tab: bool = False     # AllGather outputs in Shared addr space
    host_sel: bool = False       # DMA host-built selection matrices
                                 # instead of building them on DVE
    ag_splits: int = 4           # split each AllGather into S sub-collectives
    swdge_queues: int = 4        # SWDGE queues to round-robin gathers over
    # ablation switches (timing experiments only — results become wrong)
    abl_no_ag: bool = False      # replace AllGather with a local DMA copy
    abl_no_gather: bool = False  # replace dma_gather with DVE memset
    abl_no_mm: bool = False      # skip sel build + SpMM matmuls


FULL_CFG = GCNConfig()


# ---------------------------------------------------------------------------
# host-side preprocessing
# ---------------------------------------------------------------------------


def _wrap_idx(tok: np.ndarray) -> np.ndarray:
    """[..., n] int16 token list -> [..., 16, n//16] wrapped layout.

    dma_gather expects token i at [i % 16, i // 16] in 16 partitions.
    """
    lead = tok.shape[:-1]
    n = tok.shape[-1]
    w = tok.reshape(*lead, n // 16, 16)
    return np.swapaxes(w, -1, -2)


def preprocess(cfg: GCNConfig, x, edge_index, W1, b1, g1, be1, m1, v1,
               W2, b2, g2, be2, m2, v2, W3, b3):
    n = cfg.n
    C, NB = cfg.ncores, cfg.nb
    hid, kin = cfg.hid, cfg.kin
    k2 = hid // P

    x = np.asarray(x, dtype=np.float32)
    edge_index = np.asarray(edge_index)
    loops = np.arange(n, dtype=np.int64)
    src = np.concatenate([edge_index[0].astype(np.int64), loops])
    dst = np.concatenate([edge_index[1].astype(np.int64), loops])

    deg = np.bincount(dst, minlength=n).astype(np.float32)
    dinv = np.where(deg > 0, 1.0 / np.sqrt(deg), 0.0).astype(np.float32)
    norm = (dinv[src] * dinv[dst]).astype(np.float32)

    core = dst // cfg.shard
    blk = (dst % cfg.shard) // P
    dl = (dst % P).astype(np.float32)
    # gather-table row of a source node: with split AllGathers the table is
    # laid out (subrange, core, row) so each sub-collective output is a
    # contiguous row block. Edges are grouped by subrange so each gather
    # depends only on its own sub-collective slice.
    S = cfg.ag_splits
    sub = cfg.shard // S
    qrows = C * sub
    s_core = src // cfg.shard
    s_loc = src % cfg.shard
    q = s_loc // sub
    gval = (s_core * sub + (s_loc % sub)).astype(np.int16)  # quarter-relative

    gid = (core * NB + blk) * S + q
    ngroups = C * NB * S
    counts = np.bincount(gid, minlength=ngroups)
    cS = counts.reshape(C, NB, S)
    m_q = [max(1, int(np.ceil(cS[:, :, j].max() / P))) for j in range(S)]
    M = sum(m_q)
    moff = np.concatenate([[0], np.cumsum(m_q)])  # chunk offsets per quarter

    cap = np.empty(ngroups, dtype=np.int64)
    for j in range(S):
        cap[j::S] = m_q[j] * P
    base = np.concatenate([[0], np.cumsum(cap)])[:-1]

    order = np.argsort(gid, kind="stable")
    gs = gid[order]
    start_sorted = np.concatenate([[0], np.cumsum(counts)])[:-1]
    rank = np.arange(len(gs)) - start_sorted[gs]
    slot = base[gs] + rank

    tot = int(cap.sum())
    s_idx = np.zeros(tot, dtype=np.int16)
    s_dl = np.full(tot, -1.0, dtype=np.float32)   # pad: match no dst column
    s_idx[slot] = gval[order]
    s_dl[slot] = dl[order]

    s_idx3 = s_idx.reshape(C, NB, M * P)
    w_all = np.concatenate(
        [_wrap_idx(s_idx3[..., moff[j] * P: moff[j + 1] * P])
         for j in range(S)], axis=-1)               # [C, NB, 16, M*8]
    w_all = np.tile(w_all, (1, 1, 8, 1))            # [C, NB, 128, M*8]
    gidx_dram = np.ascontiguousarray(
        np.moveaxis(w_all, 2, 1)).reshape(C, P, NB * M * 8)

    # per-edge dst-local index, [C, P(edge slot), NB*M] layout (bf16: values
    # in [-1, 127] are exact, and 16-bit doubles DVE strip-build throughput)
    dl4 = s_dl.reshape(C, NB, M, P)
    meta_dram = np.ascontiguousarray(
        np.moveaxis(dl4, 3, 1)).astype(NPBF).reshape(C, P, NB * M)

    # host-built selection matrices: sel[e, d] = (dstlocal_e == d)
    sel5 = (dl4[..., None] == np.arange(P, dtype=np.float32)).astype(NPBF)
    selv_dram = np.ascontiguousarray(
        np.moveaxis(sel5, 3, 1)).reshape(C, P, NB * M * P)

    # per-node dinv, laid out [P, NB] per core (column nb = block's nodes)
    dinv_pad = np.zeros(cfg.npad, dtype=np.float32)
    dinv_pad[:n] = dinv
    dinvd_dram = np.ascontiguousarray(
        dinv_pad.reshape(C, NB, P).transpose(0, 2, 1))  # [C, P, NB]

    # dense inputs: x^T tiles [c, nb, ci, k*nn]
    x_pad = np.zeros((cfg.npad, kin * P), dtype=np.float32)
    x_pad[:n] = x
    xt = x_pad.reshape(C, NB, P, kin, P).transpose(0, 1, 4, 3, 2)
    xT_dram = np.ascontiguousarray(xt).astype(NPBF).reshape(C, NB, P, kin * P)

    # weights with BN scale folded in; [ci, k*co] layout
    s1 = (np.asarray(g1) / np.sqrt(np.asarray(v1) + cfg.eps)).astype(np.float32)
    s2 = (np.asarray(g2) / np.sqrt(np.asarray(v2) + cfg.eps)).astype(np.float32)
    W1p = np.asarray(W1, np.float32) * s1[None, :]
    W2p = np.asarray(W2, np.float32) * s2[None, :]

    def wtiles(w, ktiles, cout):
        t = w.reshape(ktiles, P, cout).transpose(1, 0, 2)
        return np.ascontiguousarray(t).astype(NPBF).reshape(P, ktiles * cout)

    w1_dram = wtiles(W1p, kin, hid)
    w2_dram = wtiles(W2p, k2, hid)
    w3_dram = wtiles(np.asarray(W3, np.float32), k2, cfg.out)

    bias1 = ((np.asarray(b1) - np.asarray(m1)) * s1 + np.asarray(be1)).astype(np.float32)
    bias2 = ((np.asarray(b2) - np.asarray(m2)) * s2 + np.asarray(be2)).astype(np.float32)
    bias1_bc = np.ascontiguousarray(np.broadcast_to(bias1, (P, hid)))
    bias2_bc = np.ascontiguousarray(np.broadcast_to(bias2, (P, hid)))
    b3_bc = np.ascontiguousarray(
        np.broadcast_to(np.asarray(b3, np.float32), (P, cfg.out)))

    iota = np.ascontiguousarray(
        np.broadcast_to(np.arange(P, dtype=np.float32), (P, P))).astype(NPBF)
    ident = np.eye(P, dtype=np.float32).astype(NPBF)

    in_maps = []
    for c in range(C):
        extra = ({"selv": selv_dram[c]} if cfg.host_sel
                 else {"meta": meta_dram[c]})
        in_maps.append({
            **extra,
            "xT": xT_dram[c],
            "gidx": gidx_dram[c],
            "dinvd": dinvd_dram[c],
            "w1": w1_dram,
            "w2": w2_dram,
            "w3": w3_dram,
            "bias1": bias1_bc,
            "bias2": bias2_bc,
            "b3bc": b3_bc,
            "iota": iota,
            "ident": ident,
        })
    return in_maps, tuple(m_q)


# ---------------------------------------------------------------------------
# device kernel
# ---------------------------------------------------------------------------


def build_nc(cfg: GCNConfig, m_q: tuple):
    C, NB = cfg.ncores, cfg.nb
    hid, kin, outd = cfg.hid, cfg.kin, cfg.out
    k2 = hid // P
    S = cfg.ag_splits
    sub = cfg.shard // S
    qrows = C * sub
    M = sum(m_q)
    moff = [0]
    for m in m_q:
        moff.append(moff[-1] + m)

    nc = bacc.Bacc("TRN2", target_bir_lowering=False, debug=False,
                   num_devices=C, dynamic_dma_scratch_size=cfg.dma_scratch,
                   num_swdge_queues=cfg.swdge_queues)

    xT = nc.dram_tensor("xT", [NB, P, kin * P], BF16, kind="ExternalInput")
    gidx = nc.dram_tensor("gidx", [P, NB * M * 8], I16, kind="ExternalInput")
    if cfg.host_sel:
        selv = nc.dram_tensor("selv", [P, NB * M * P], BF16,
                              kind="ExternalInput")
    else:
        meta = nc.dram_tensor("meta", [P, NB * M], BF16,
                              kind="ExternalInput")
    w1 = nc.dram_tensor("w1", [P, kin * hid], BF16, kind="ExternalInput")
    w2 = nc.dram_tensor("w2", [P, k2 * hid], BF16, kind="ExternalInput")
    w3 = nc.dram_tensor("w3", [P, k2 * outd], BF16, kind="ExternalInput")
    dinvd = nc.dram_tensor("dinvd", [P, NB], F32, kind="ExternalInput")
    bias1 = nc.dram_tensor("bias1", [P, hid], F32, kind="ExternalInput")
    bias2 = nc.dram_tensor("bias2", [P, hid], F32, kind="ExternalInput")
    b3bc = nc.dram_tensor("b3bc", [P, outd], F32, kind="ExternalInput")
    iota = nc.dram_tensor("iota", [P, P], BF16, kind="ExternalInput")
    ident = nc.dram_tensor("ident", [P, P], BF16, kind="ExternalInput")
    out = nc.dram_tensor("out", [cfg.shard, outd], F32, kind="ExternalOutput")

    groups = [list(range(C))]

    with tile.TileContext(nc) as tc:
        with (
            tc.tile_pool(name="const", bufs=1) as constp,
            tc.tile_pool(name="zfeat", bufs=1) as zfeatp,
            tc.tile_pool(name="dram", bufs=2, space="DRAM") as dramp,
            tc.tile_pool(name="work", bufs=3) as workp,
            tc.tile_pool(name="msg", bufs=2) as msgp,
            tc.tile_pool(name="sel", bufs=2) as selp,
            tc.tile_pool(name="psum", bufs=3, space="PSUM") as psump,
            tc.tile_pool(name="psumt", bufs=2, space="PSUM") as psumtp,
        ):
            # ---- constants ----
            w1_t = constp.tile([P, kin * hid], BF16, tag="w1")
            nc.sync.dma_start(out=w1_t[:], in_=w1[:, :])
            w2_t = constp.tile([P, k2 * hid], BF16, tag="w2")
            nc.sync.dma_start(out=w2_t[:], in_=w2[:, :])
            w3_t = constp.tile([P, k2 * outd], BF16, tag="w3")
            nc.sync.dma_start(out=w3_t[:], in_=w3[:, :])
            bias1_t = constp.tile([P, hid], F32, tag="bias1")
            nc.sync.dma_start(out=bias1_t[:], in_=bias1[:, :])
            bias2_t = constp.tile([P, hid], F32, tag="bias2")
            nc.sync.dma_start(out=bias2_t[:], in_=bias2[:, :])
            b3_t = constp.tile([P, outd], F32, tag="b3")
            nc.sync.dma_start(out=b3_t[:], in_=b3bc[:, :])
            iota_t = constp.tile([P, P], BF16, tag="iota")
            nc.sync.dma_start(out=iota_t[:], in_=iota[:, :])
            ident_t = constp.tile([P, P], BF16, tag="ident")
            nc.sync.dma_start(out=ident_t[:], in_=ident[:, :])
            gidx_t = constp.tile([P, NB * M * 8], I16, tag="gidx")
            nc.sync.dma_start(out=gidx_t[:], in_=gidx[:, :])
            dinvd_t = constp.tile([P, NB], F32, tag="dinvd")
            nc.sync.dma_start(out=dinvd_t[:], in_=dinvd[:, :])
            if not cfg.host_sel:
                meta_t = constp.tile([P, NB * M], BF16, tag="meta")
                nc.sync.dma_start(out=meta_t[:], in_=meta[:, :])

            # persistent feature-major activations ([feat, k * shard] layout)
            def new_zfeat(name):
                return zfeatp.tile([P, k2 * cfg.shard], BF16, tag="zf",
                                   name=name)

            def allgather(src_tile, cols, name):
                # one table tile per sub-collective so each has a single
                # writer (required for Shared addr space) and gathers depend
                # only on their own slice
                space = "Shared" if cfg.shared_tab else "Local"
                tabs = []
                for q in range(S):
                    tab = dramp.tile([qrows, cols], BF16, tag=f"tab{q}",
                                     name=f"{name}_q{q}", addr_space=space)
                    if cfg.abl_no_ag:
                        nc.sync.dma_start(
                            out=tab[:sub, :],
                            in_=src_tile[q * sub:(q + 1) * sub, :])
                    else:
                        nc.gpsimd.collective_compute(
                            "AllGather",
                            mybir.AluOpType.bypass,
                            replica_groups=groups,
                            ins=[src_tile[q * sub:(q + 1) * sub, :].opt()],
                            outs=[tab[:, :].opt()],
                        )
                    tabs.append(tab)
                return tabs

            # ---- dense matmul phase: h_sh[nb*P:+P] = lhs(nb) @ W ----
            def dense(lhs_of_nb, w_tile, ktiles, cols, epilogue):
                for nb in range(NB):
                    ps = psump.tile([P, cols], F32, tag="mm", name="dps")
                    for k in range(ktiles):
                        nc.tensor.matmul(
                            out=ps[:],
                            lhsT=lhs_of_nb(nb, k),
                            rhs=w_tile[:, k * cols:(k + 1) * cols],
                            start=(k == 0),
                            stop=(k == ktiles - 1),
                        )
                    epilogue(nb, ps)

            # ---- sparse propagation: psum[dst 128, hid] = A_nb @ table ----
            gq_counter = [0]

            def next_gq():
                q = gq_counter[0] % cfg.swdge_queues
                gq_counter[0] += 1
                return q

            def spmm(tabs, epilogue, lname):
                G = cfg.max_gather_chunks
                for nb in range(NB):
                    gbase = nb * M * 8
                    mbase = nb * M
                    g_all = msgp.tile([P, M, hid], BF16, tag="gall",
                                      name=f"gall_{lname}")
                    if cfg.abl_no_gather:
                        nc.vector.memset(g_all[:], 0.0)
                    else:
                        for j in range(S):
                            for g0 in range(moff[j], moff[j + 1], G):
                                g1 = min(g0 + G, moff[j + 1])
                                nc.gpsimd.dma_gather(
                                    out_ap=g_all[:, g0:g1, :],
                                    in_ap=tabs[j][:, :],
                                    idxs_ap=gidx_t[:, gbase + g0 * 8:
                                                   gbase + g1 * 8],
                                    num_idxs=(g1 - g0) * P,
                                    num_idxs_reg=(g1 - g0) * P,
                                    elem_size=hid,
                                    single_packet=cfg.single_packet,
                                    queue_num=next_gq(),
                                )
                    ps = psump.tile([P, hid], F32, tag="mm", name="sps")
                    if cfg.abl_no_mm:
                        nc.vector.memset(ps[:], 0.0)
                        # keep a data dep on the gathers so they stay live
                        nc.vector.tensor_tensor(
                            out=ps[:, :1], in0=g_all[:, 0, :1],
                            in1=g_all[:, M - 1, :1], op=mybir.AluOpType.add)
                        epilogue(nb, ps)
                        continue
                    strip = selp.tile([P, M * P], BF16, tag="selb",
                                      name="selstrip")
                    if cfg.host_sel:
                        nc.sync.dma_start(
                            out=strip[:],
                            in_=selv[:, nb * M * P:(nb + 1) * M * P])
                    else:
                        # one DVE op builds all M one-hot sel matrices:
                        # strip[e, m*128+d] = (dstlocal[e, m] == d)
                        dl_b = meta_t[:, mbase:mbase + M] \
                            .unsqueeze(-1).broadcast_to([P, M, P])
                        io_b = iota_t[:, :].unsqueeze(1) \
                            .broadcast_to([P, M, P])
                        nc.vector.tensor_tensor(
                            out=strip[:].rearrange("p (m d) -> p m d", m=M),
                            in0=dl_b, in1=io_b,
                            op=mybir.AluOpType.is_equal)
                    for m in range(M):
                        nc.tensor.matmul(
                            out=ps[:],
                            lhsT=strip[:, m * P:(m + 1) * P],
                            rhs=g_all[:, m, :],
                            start=(m == 0),
                            stop=(m == M - 1),
                        )
                    epilogue(nb, ps)

            # node-major [128 nodes, hid] sbuf tile -> feature-major zT slices
            def to_featmajor(zT_t, nb, zn):
                for f in range(k2):
                    pt = psumtp.tile([P, P], BF16, tag="pt", name="pt")
                    nc.tensor.transpose(
                        out=pt[:], in_=zn[:, f * P:(f + 1) * P],
                        identity=ident_t[:])
                    nc.scalar.copy(
                        out=zT_t[:, f * cfg.shard + nb * P:
                                 f * cfg.shard + (nb + 1) * P],
                        in_=pt[:])

            def bn_relu(nb, ps, bias_t, name):
                # z = relu(dinv[dst] * psum + bias)
                sc = workp.tile([P, hid], F32, tag="epsc", name="epsc")
                nc.vector.tensor_scalar_mul(
                    out=sc[:], in0=ps[:], scalar1=dinvd_t[:, nb:nb + 1])
                tmp = workp.tile([P, hid], F32, tag="eptmp", name="eptmp")
                nc.vector.tensor_tensor(out=tmp[:], in0=sc[:], in1=bias_t[:],
                                        op=mybir.AluOpType.add)
                zn = workp.tile([P, hid], BF16, tag="zn", name=name)
                nc.vector.tensor_scalar_max(out=zn[:], in0=tmp[:], scalar1=0.0)
                return zn

            # ================= layer 1 =================
            h_sh1 = dramp.tile([cfg.shard, hid], BF16, tag="hsh", name="h_sh1")

            def ep_dense1(nb, ps):
                hs = workp.tile([P, hid], BF16, tag="hs", name="hs1")
                nc.vector.tensor_scalar_mul(
                    out=hs[:], in0=ps[:], scalar1=dinvd_t[:, nb:nb + 1])
                nc.sync.dma_start(out=h_sh1[nb * P:(nb + 1) * P, :], in_=hs[:])

            # L1 dense streams xT tiles from DRAM
            for nb in range(NB):
                xt_t = workp.tile([P, kin * P], BF16, tag="xt", name="xt")
                nc.sync.dma_start(out=xt_t[:], in_=xT[nb, :, :])
                ps = psump.tile([P, hid], F32, tag="mm", name="dps1")
                for k in range(kin):
                    nc.tensor.matmul(
                        out=ps[:],
                        lhsT=xt_t[:, k * P:(k + 1) * P],
                        rhs=w1_t[:, k * hid:(k + 1) * hid],
                        start=(k == 0),
                        stop=(k == kin - 1),
                    )
                ep_dense1(nb, ps)

            h_tab1 = allgather(h_sh1, hid, "h_tab1")

            z1T = new_zfeat("z1T")

            def ep_spmm1(nb, ps):
                zn = bn_relu(nb, ps, bias1_t, "zn1")
                to_featmajor(z1T, nb, zn)

            spmm(h_tab1, ep_spmm1, "l1")

            # ================= layer 2 =================
            h_sh2 = dramp.tile([cfg.shard, hid], BF16, tag="hsh", name="h_sh2")

            def ep_dense2(nb, ps):
                hs = workp.tile([P, hid], BF16, tag="hs", name="hs2")
                nc.vector.tensor_scalar_mul(
                    out=hs[:], in0=ps[:], scalar1=dinvd_t[:, nb:nb + 1])
                nc.sync.dma_start(out=h_sh2[nb * P:(nb + 1) * P, :], in_=hs[:])

            dense(lambda nb, k: z1T[:, k * cfg.shard + nb * P:
                                    k * cfg.shard + (nb + 1) * P],
                  w2_t, k2, hid, ep_dense2)

            h_tab2 = allgather(h_sh2, hid, "h_tab2")

            z_sh2 = dramp.tile([cfg.shard, hid], BF16, tag="hsh", name="z_sh2")

            def ep_spmm2(nb, ps):
                zn = bn_relu(nb, ps, bias2_t, "zn2")
                zs = workp.tile([P, hid], BF16, tag="zs", name="zs2")
                nc.vector.tensor_scalar_mul(
                    out=zs[:], in0=zn[:], scalar1=dinvd_t[:, nb:nb + 1])
                nc.sync.dma_start(out=z_sh2[nb * P:(nb + 1) * P, :], in_=zs[:])

            spmm(h_tab2, ep_spmm2, "l2")

            z_tab2 = allgather(z_sh2, hid, "z_tab2")

            # ================= layer 3 =================
            p3T = new_zfeat("p3T")

            def ep_spmm3(nb, ps):
                pn = workp.tile([P, hid], BF16, tag="zn", name="pn3")
                nc.vector.tensor_scalar_mul(
                    out=pn[:], in0=ps[:], scalar1=dinvd_t[:, nb:nb + 1])
                to_featmajor(p3T, nb, pn)

            spmm(z_tab2, ep_spmm3, "l3")

            def ep_dense3(nb, ps):
                ot = workp.tile([P, outd], F32, tag="ot", name="ot")
                nc.vector.tensor_tensor(out=ot[:], in0=ps[:], in1=b3_t[:],
                                        op=mybir.AluOpType.add)
                nc.sync.dma_start(out=out[nb * P:(nb + 1) * P, :], in_=ot[:])

            dense(lambda nb, k: p3T[:, k * cfg.shard + nb * P:
                                    k * cfg.shard + (nb + 1) * P],
                  w3_t, k2, outd, ep_dense3)

    nc.compile()
    return nc


# ---------------------------------------------------------------------------
# entry point
# ---------------------------------------------------------------------------

_NC_CACHE: dict = {}


def prepare(cfg: GCNConfig, inputs: dict):
    in_maps, m_q = preprocess(cfg, **inputs)
    key = (cfg, m_q)
    if key not in _NC_CACHE:
        _NC_CACHE[key] = build_nc(cfg, m_q)
    return _NC_CACHE[key], in_maps


def run(cfg: GCNConfig, inputs: dict, **spmd_kwargs):
    nc, in_maps = prepare(cfg, inputs)
    res = run_bass_kernel_spmd(nc, in_maps, core_ids=list(range(cfg.ncores)),
                               **spmd_kwargs)
    full = np.concatenate(
        [res.results[c]["out"] for c in range(cfg.ncores)], axis=0)
    return np.ascontiguousarray(full[:cfg.n]).astype(np.float32), res


def kernel(**inputs) -> np.ndarray:
    out, _ = run(FULL_CFG, inputs)
    return out

